# revision 6
# baseline (speedup 1.0000x reference)
"""MoE routing kernel for Trainium2 (8 NeuronCores, expert-parallel).

Problem (hardcoded shapes): B=4, S=2048, H=1024, I=4096, E=8, capacity=1024.

Mathematical simplification of the reference: softmax routing weights are
strictly positive, so the routing mask is all-ones and the stable argsort of
the (constant) mask is the identity permutation.  Consequently every expert
processes exactly tokens 0..1023 of the flattened [8192, 1024] input, and the
output is nonzero only for those tokens:

    out[n] = sum_e softmax(x[n] @ Wr.T + b)[e] * (relu(x[n] @ Wi[e]) @ Wo[e])

Sharding: expert-parallel.  Each of the 8 cores receives the same 1024-token
slice (pre-transposed to X^T on host) and the weights of ONE expert; it
computes that expert's weighted output [1024, 1024].  The host gathers the 8
partial outputs, sums them (the MoE combine), and scatters into the full
[4, 2048, 1024] zero tensor.

Per-core device computation (all fp32, matmuls in float32r = fp22 precision,
full PE rate at moving-dim >= 256):
  router:   logits = X @ Wr_perm.T + b_perm  (expert column 0 == own expert)
            w_e = softmax(logits)[:, 0]
  layer 1:  inter^T[I, tok] = relu(Wi^T X^T)    (two 512-token chunks)
  layer 2:  out[tok, H] = (inter^T).T @ Wo, scaled by w_e per token
"""

import numpy as np

_CACHE = {}

B, S, H, I, E = 4, 2048, 1024, 4096, 8
CAP = 1024  # capacity = ceil(B*S/E)
N_CORES = 8


def _build():
    import concourse.bacc as bacc
    import concourse.mybir as mybir
    import concourse.tile as tile

    f32 = mybir.dt.float32
    f32r = mybir.dt.float32r
    AF = mybir.ActivationFunctionType
    AX = mybir.AxisListType

    nc = bacc.Bacc("TRN2", target_bir_lowering=False, debug=False)

    xt_d = nc.dram_tensor("xt", [H, CAP], f32r, kind="ExternalInput")
    rwtb_d = nc.dram_tensor("rwtb", [H + 1, E], f32r, kind="ExternalInput")
    wi_d = nc.dram_tensor("wi", [H, I], f32r, kind="ExternalInput")
    wo_d = nc.dram_tensor("wo", [I, H], f32r, kind="ExternalInput")
    out_d = nc.dram_tensor("out", [CAP, H], f32, kind="ExternalOutput")

    KT = H // 128   # 8 k-tiles (H on partitions)
    IT = I // 128   # 32 I-tiles
    MT = CAP // 128  # 8 token tiles

    with tile.TileContext(nc) as tc:
        with (
            tc.tile_pool(name="const", bufs=1) as const_pool,
            tc.tile_pool(name="wi", bufs=3) as wi_pool,
            tc.tile_pool(name="wo", bufs=3) as wo_pool,
            tc.tile_pool(name="inter", bufs=1) as inter_pool,
            tc.tile_pool(name="outs", bufs=4) as outs_pool,
            tc.tile_pool(name="small", bufs=4) as small_pool,
            tc.tile_pool(name="psl", bufs=2, space="PSUM") as psum_log,
            tc.tile_pool(name="ps1", bufs=2, space="PSUM") as psum1,
            tc.tile_pool(name="ps2", bufs=1, space="PSUM") as psum2,
        ):
            # ---- resident tensors ----
            xt_sb = const_pool.tile([128, KT, CAP], f32r)
            nc.sync.dma_start(
                xt_sb[:], xt_d.ap().rearrange("(t p) n -> p t n", p=128)
            )
            rwt_sb = const_pool.tile([128, KT, E], f32r)
            nc.sync.dma_start(
                rwt_sb[:], rwtb_d.ap()[0:H, :].rearrange("(t p) e -> p t e", p=128)
            )
            b_sb = const_pool.tile([1, E], f32r)
            nc.sync.dma_start(b_sb[:], rwtb_d.ap()[H : H + 1, :])
            ones_f = const_pool.tile([1, 128], f32)
            nc.vector.memset(ones_f[:], 1.0)
            ones_sb = const_pool.tile([1, 128], f32r)
            nc.vector.tensor_copy(ones_sb[:], ones_f[:])
            wgt_sb = const_pool.tile([128, MT], f32)

            # ---- router: per-token softmax weight of own expert (col 0) ----
            for m in range(MT):
                lg = psum_log.tile([128, E], f32)
                for k in range(KT):
                    nc.tensor.matmul(
                        lg[:],
                        xt_sb[:, k, m * 128 : (m + 1) * 128],
                        rwt_sb[:, k, :],
                        start=(k == 0),
                        stop=False,
                    )
                # add router bias (broadcast row): ones[1,128].T @ b[1,E]
                nc.tensor.matmul(
                    lg[:],
                    ones_sb[:, :],
                    b_sb[:, :],
                    start=False,
                    stop=True,
                )
                nmx = small_pool.tile([128, 1], f32, tag="nmx")
                nc.vector.reduce_max(nmx[:], lg[:], axis=AX.X, negate=True)
                ex = small_pool.tile([128, E], f32, tag="ex")
                nc.scalar.activation(ex[:], lg[:], AF.Exp, bias=nmx[:])
                sm = small_pool.tile([128, 1], f32, tag="sm")
                nc.vector.reduce_sum(sm[:], ex[:], axis=AX.X)
                rc = small_pool.tile([128, 1], f32, tag="rc")
                nc.vector.reciprocal(rc[:], sm[:])
                nc.vector.tensor_scalar_mul(
                    wgt_sb[:, m : m + 1], ex[:, 0:1], rc[:]
                )

            # ---- expert FFN over two 512-token chunks ----
            for c in range(2):
                t0 = c * 512
                inter = inter_pool.tile([128, IT, 512], f32r)
                # layer 1: inter^T[it] = relu(sum_k wi[k,it].T @ xt[k, tok])
                for it in range(IT):
                    wi_t = wi_pool.tile([128, KT, 128], f32r)
                    nc.sync.dma_start(
                        wi_t[:],
                        wi_d.ap()[:, it * 128 : (it + 1) * 128].rearrange(
                            "(k p) i -> p k i", p=128
                        ),
                    )
                    p1 = psum1.tile([128, 512], f32)
                    for k in range(KT):
                        nc.tensor.matmul(
                            p1[:],
                            wi_t[:, k, :],
                            xt_sb[:, k, t0 : t0 + 512],
                            start=(k == 0),
                            stop=(k == KT - 1),
                        )
                    nc.scalar.activation(inter[:, it, :], p1[:], AF.Relu)

                # layer 2: out[tok, hch] = sum_it inter^T[it,tok].T @ wo[it,hch]
                for hch in range(2):
                    p2s = [
                        psum2.tile([128, 512], f32, tag=f"p2_{t}", name=f"p2_{t}")
                        for t in range(4)
                    ]
                    for it in range(IT):
                        wo_t = wo_pool.tile([128, 512], f32r)
                        nc.sync.dma_start(
                            wo_t[:],
                            wo_d.ap()[
                                it * 128 : (it + 1) * 128,
                                hch * 512 : (hch + 1) * 512,
                            ],
                        )
                        for tsub in range(4):
                            nc.tensor.matmul(
                                p2s[tsub][:],
                                inter[:, it, tsub * 128 : (tsub + 1) * 128],
                                wo_t[:],
                                start=(it == 0),
                                stop=(it == IT - 1),
                            )
                    for tsub in range(4):
                        m = c * 4 + tsub
                        o = outs_pool.tile([128, 512], f32)
                        nc.vector.tensor_scalar_mul(
                            o[:], p2s[tsub][:], wgt_sb[:, m : m + 1]
                        )
                        nc.sync.dma_start(
                            out_d.ap()[
                                m * 128 : (m + 1) * 128,
                                hch * 512 : (hch + 1) * 512,
                            ],
                            o[:],
                        )

    nc.compile()
    return nc


def get_nc():
    if "nc" not in _CACHE:
        _CACHE["nc"] = _build()
    return _CACHE["nc"]


def make_in_maps(x, router_w, router_b, experts_inter, experts_out):
    x_flat = np.asarray(x, dtype=np.float32).reshape(-1, H)
    xt = np.ascontiguousarray(x_flat[:CAP].T)
    in_maps = []
    for e in range(N_CORES):
        perm = [e] + [j for j in range(E) if j != e]
        rw = np.asarray(router_w, dtype=np.float32)[perm]  # [E, H]
        rb = np.asarray(router_b, dtype=np.float32)[perm]  # [E]
        rwtb = np.concatenate([rw.T, rb[None, :]], axis=0)  # [H+1, E]
        in_maps.append(
            {
                "xt": xt,
                "rwtb": np.ascontiguousarray(rwtb),
                "wi": np.ascontiguousarray(experts_inter[e], dtype=np.float32),
                "wo": np.ascontiguousarray(experts_out[e], dtype=np.float32),
            }
        )
    return in_maps


def combine(results):
    partial = np.zeros((CAP, H), dtype=np.float32)
    for r in results:
        partial += r["out"]
    out = np.zeros((B * S, H), dtype=np.float32)
    out[:CAP] = partial
    return out.reshape(B, S, H)


def kernel(x, router_w, router_b, experts_inter, experts_out):
    from concourse import bass_utils

    nc = get_nc()
    in_maps = make_in_maps(x, router_w, router_b, experts_inter, experts_out)
    res = bass_utils.run_bass_kernel_spmd(nc, in_maps, core_ids=list(range(N_CORES)))
    return combine(res.results)
